# revision 14
# baseline (speedup 1.0000x reference)
"""DeltaNet model kernel for Trainium2, 8-core data-parallel over batch.

Model (per batch b): input-proj -> fast_ff(0) -> ffn -> fast_ff(1), where
fast_ff = LN -> qkvb proj -> softmax(q,k), sigmoid(beta) -> delta-rule
attention -> out proj -> residual.

The delta-rule scan (W_t = W_{t-1} + beta_t (v_t - W_{t-1} k_t) k_t^T) is
computed chunkwise (C=128) via the UT/WY transform:
  A   = strict_tril(diag(beta) K K^T)            [C,C]
  X   = (I+A)^{-1} [beta*V | beta*K]             (truncated Neumann, order 4:
                                                  X = (I-A)(I+A^2) R)
  M   = tril_incl(Q K^T)
  O_c = (Q - M Tk) W0^T + M Tv
  W1  = W0 + Tv^T K - W0 (Tk^T K)
with [Tv|Tk] = X. Only the W recurrence (4 small matmuls/chunk/head) is
sequential; everything else is chunk-parallel. All matmuls in bf16 with fp32
PSUM accumulation; the residual stream and LN/softmax statistics stay fp32.

Layouts: token-major (TM) = [token partitions, feature free];
feature-major (FM) = [feature partitions, token free]. Per-head parity
bh = (h%2)*64 places head h's FM slices at partition base bh (PE matmuls
require lhsT/rhs on the same base partition).
"""
import numpy as np
import ml_dtypes
from contextlib import ExitStack

bfnp = ml_dtypes.bfloat16

# model dims
S, HID, H, D, FF = 2048, 512, 8, 64, 2048
C = 128               # delta chunk == token tile
EPS = 1e-5
N_CORES = 8
NEUMANN8 = False      # order-8 solve instead of order-4

_CACHED = {}


def build(nc_cls, s=S, n_cores=N_CORES, dbg=False):
    import concourse.bass as bass
    import concourse.tile as tile
    from concourse import mybir

    f32 = mybir.dt.float32
    bft = mybir.dt.bfloat16
    ts = bass.ts
    NT = s // 128          # token tiles / chunks
    act = mybir.ActivationFunctionType
    alu = mybir.AluOpType

    nc = nc_cls("TRN2", target_bir_lowering=False, debug=False,
                num_devices=n_cores)

    # ---- dram I/O ----
    xT_d = nc.dram_tensor("xT", [HID, s], bft, kind="ExternalInput")
    ipwT_d = nc.dram_tensor("ipwT", [HID, HID], bft, kind="ExternalInput")
    ipb_d = nc.dram_tensor("ipb_bc", [128, HID], f32, kind="ExternalInput")
    swT_d = nc.dram_tensor("swT", [2, HID, 1544], bft, kind="ExternalInput")
    owT_d = nc.dram_tensor("owT", [2, HID, HID], bft, kind="ExternalInput")
    w1T_d = nc.dram_tensor("w1T", [HID, FF], bft, kind="ExternalInput")
    b1_d = nc.dram_tensor("b1c", [16, 128, 1], f32, kind="ExternalInput")
    w2T_d = nc.dram_tensor("w2T", [FF, HID], bft, kind="ExternalInput")
    b2_d = nc.dram_tensor("b2_bc", [128, HID], f32, kind="ExternalInput")
    # LN params broadcast to [128, HID]; order: fw0, ff, fw1
    lng_d = nc.dram_tensor("lng_bc", [3, 128, HID], f32, kind="ExternalInput")
    lnb_d = nc.dram_tensor("lnb_bc", [3, 128, HID], f32, kind="ExternalInput")
    # masks: 0 = strict lower, 1 = strict upper, 2 = incl upper   [128,128] f32
    msk_d = nc.dram_tensor("masks", [3, 128, 128], f32, kind="ExternalInput")
    idn_d = nc.dram_tensor("ident", [128, 128], bft, kind="ExternalInput")
    out_d = nc.dram_tensor("out", [s, HID], f32, kind="ExternalOutput")
    if dbg:
        dbg_r0 = nc.dram_tensor("dbg_r0", [s, HID], f32, kind="ExternalOutput")
        dbg_h = nc.dram_tensor("dbg_h", [s, HID], f32, kind="ExternalOutput")
        dbg_q = nc.dram_tensor("dbg_q", [128, 512], f32, kind="ExternalOutput")
        dbg_k = nc.dram_tensor("dbg_k", [128, 512], f32, kind="ExternalOutput")
        dbg_kb = nc.dram_tensor("dbg_kb", [128, 512], f32, kind="ExternalOutput")
        dbg_v = nc.dram_tensor("dbg_v", [128, 512], f32, kind="ExternalOutput")
        dbg_bet = nc.dram_tensor("dbg_bet", [128, 8], f32, kind="ExternalOutput")
        dbg_attn = nc.dram_tensor("dbg_attn", [4, 128, s], f32, kind="ExternalOutput")
        dbg_r1 = nc.dram_tensor("dbg_r1", [s, HID], f32, kind="ExternalOutput")
        dbg_r2 = nc.dram_tensor("dbg_r2", [s, HID], f32, kind="ExternalOutput")

    with tile.TileContext(nc) as tc, ExitStack() as ctx:
        wp = ctx.enter_context(tc.tile_pool(name="wts", bufs=1))
        cp = ctx.enter_context(tc.tile_pool(name="const", bufs=1))
        rp = ctx.enter_context(tc.tile_pool(name="res", bufs=1))
        ap = ctx.enter_context(tc.tile_pool(name="acts", bufs=2))
        ip = ctx.enter_context(tc.tile_pool(name="inst", bufs=3))
        sp = ctx.enter_context(tc.tile_pool(name="state", bufs=2))
        anp = ctx.enter_context(tc.tile_pool(name="attn", bufs=1))
        ffp = ctx.enter_context(tc.tile_pool(name="ffn", bufs=1))
        ps_big = ctx.enter_context(
            tc.tile_pool(name="psbig", bufs=2, space=bass.MemorySpace.PSUM))
        ps_wrk = ctx.enter_context(
            tc.tile_pool(name="pswrk", bufs=3, space=bass.MemorySpace.PSUM))
        ps_ss = ctx.enter_context(
            tc.tile_pool(name="psss", bufs=2, space=bass.MemorySpace.PSUM))
        ps_tp = ctx.enter_context(
            tc.tile_pool(name="pstp", bufs=1, space=bass.MemorySpace.PSUM))

        # ---- consts ----
        ident = cp.tile([128, 128], bft, tag="ident", name="ident")
        nc.sync.dma_start(ident[:], idn_d[:])
        mlo = cp.tile([128, 128], f32, tag="mlo", name="mlo")
        mup = cp.tile([128, 128], f32, tag="mup", name="mup")
        mui = cp.tile([128, 128], f32, tag="mui", name="mui")
        nc.sync.dma_start(mlo[:], msk_d[0])
        nc.sync.dma_start(mup[:], msk_d[1])
        nc.sync.dma_start(mui[:], msk_d[2])
        lng = [cp.tile([128, HID], f32, tag=f"lng{i}", name=f"lng{i}") for i in range(3)]
        lnb = [cp.tile([128, HID], f32, tag=f"lnb{i}", name=f"lnb{i}") for i in range(3)]
        for i in range(3):
            nc.sync.dma_start(lng[i][:], lng_d[i])
            nc.sync.dma_start(lnb[i][:], lnb_d[i])
        ipb = cp.tile([128, HID], f32, tag="ipb", name="ipb")
        nc.sync.dma_start(ipb[:], ipb_d[:])
        b2 = cp.tile([128, HID], f32, tag="b2", name="b2")
        nc.sync.dma_start(b2[:], b2_d[:])
        b1c = [cp.tile([128, 1], f32, tag=f"b1c{i}", name=f"b1c{i}") for i in range(16)]
        for i in range(16):
            nc.sync.dma_start(b1c[i][:], b1_d[i])
        epsc = cp.tile([128, 1], f32, tag="epsc", name="epsc")
        nc.gpsimd.memset(epsc[:], EPS)

        # ---- weights ----
        xTt = [wp.tile([128, s], bft, tag=f"xT{f}", name=f"xT{f}") for f in range(4)]
        ipw = [wp.tile([128, HID], bft, tag=f"ipw{f}", name=f"ipw{f}") for f in range(4)]
        for f in range(4):
            nc.sync.dma_start(xTt[f][:], xT_d[ts(f, 128), :])
            nc.sync.dma_start(ipw[f][:], ipwT_d[ts(f, 128), :])
        swt = [wp.tile([128, 1544], bft, tag=f"swt{f}", name=f"swt{f}") for f in range(4)]
        owt = [wp.tile([128, HID], bft, tag=f"owt{f}", name=f"owt{f}") for f in range(4)]
        w1t = [wp.tile([128, FF], bft, tag=f"w1t{f}", name=f"w1t{f}") for f in range(4)]
        w2t = [wp.tile([128, HID], bft, tag=f"w2t{f}", name=f"w2t{f}") for f in range(16)]
        for f in range(4):
            nc.sync.dma_start(swt[f][:], swT_d[0, ts(f, 128), :])
            nc.sync.dma_start(owt[f][:], owT_d[0, ts(f, 128), :])
            nc.sync.dma_start(w1t[f][:], w1T_d[ts(f, 128), :])
        for f in range(16):
            nc.sync.dma_start(w2t[f][:], w2T_d[ts(f, 128), :])

        # ---- residual: input proj ----
        rt = [rp.tile([128, HID], f32, tag=f"r{i}", name=f"r{i}") for i in range(NT)]
        for i in range(NT):
            pb = ps_big.tile([128, HID], f32, tag="big", name="big")
            for f in range(4):
                nc.tensor.matmul(pb[:], xTt[f][:, ts(i, 128)], ipw[f][:],
                                 start=(f == 0), stop=(f == 3))
            nc.vector.tensor_add(rt[i][:], pb[:], ipb[:])
            if dbg:
                nc.sync.dma_start(dbg_r0[ts(i, 128), :], rt[i][:])

        def layer_norm(i, r_tile, g, b, h_out):
            """h_out (bf16 TM) = LN(r_tile) * g + b"""
            st = ap.tile([128, 8], f32, tag="lnstat", name="lnstat")
            scr = ap.tile([128, HID], f32, tag="lnscr", name="lnscr")
            nc.vector.tensor_reduce(st[:, 0:1], r_tile[:],
                                    mybir.AxisListType.X, alu.add)
            nc.scalar.activation(scr[:], r_tile[:], act.Square,
                                 accum_out=st[:, 1:2])
            nc.scalar.mul(st[:, 2:3], st[:, 0:1], 1.0 / HID)     # m
            nc.scalar.mul(st[:, 3:4], st[:, 1:2], 1.0 / HID)     # E[x^2]
            nc.vector.tensor_tensor(st[:, 4:5], st[:, 2:3], st[:, 2:3],
                                    alu.mult)                     # m^2
            nc.vector.tensor_sub(st[:, 5:6], st[:, 3:4], st[:, 4:5])  # var
            nc.scalar.activation(st[:, 6:7], st[:, 5:6], act.Sqrt, bias=epsc[:])
            nc.vector.reciprocal(st[:, 7:8], st[:, 6:7])          # rstd
            nc.vector.tensor_scalar(scr[:], r_tile[:], st[:, 2:3], st[:, 7:8],
                                    alu.subtract, alu.mult)
            nc.gpsimd.tensor_mul(scr[:], scr[:], g[:])
            nc.gpsimd.tensor_add(h_out[:], scr[:], b[:])

        def transpose4(src_aps, dst_tile):
            """PE-transpose four [128,128] bf16 APs into dst_tile [128,512]."""
            tp = ps_tp.tile([128, 512], bft, tag="tp", name="tp")
            for j, a in enumerate(src_aps):
                nc.tensor.transpose(tp[:, ts(j, 128)], a, ident[:])
            nc.scalar.copy(dst_tile[:], tp[:, 0:len(src_aps) * 128])

        def fast_ff(li, lnidx):
            attn = [anp.tile([128, s], bft, tag=f"attn{g}", name=f"attn{g}") for g in range(4)]
            if li > 0:
                for f in range(4):
                    nc.sync.dma_start(swt[f][:], swT_d[li, ts(f, 128), :])
                    nc.sync.dma_start(owt[f][:], owT_d[li, ts(f, 128), :])
            wtf = {}
            wtb = {}
            for c in range(NT):
                # LN + transpose h to FM
                h_tm = ap.tile([128, HID], bft, tag="h_tm", name="h_tm")
                layer_norm(c, rt[c], lng[lnidx], lnb[lnidx], h_tm)
                if dbg and li == 0:
                    nc.gpsimd.dma_start(dbg_h[ts(c, 128), :], h_tm[:])
                hF = ap.tile([128, HID], bft, tag="hF", name="hF")
                transpose4([h_tm[:, ts(f, 128)] for f in range(4)], hF)

                # qkvb projection (cols: q 0:512, k 512:1024, v 1024:1536,
                # beta 1536:1544 -- host permuted)
                pq = ps_big.tile([128, 512], f32, tag="big", name="big")
                pk = ps_big.tile([128, 512], f32, tag="big", name="big")
                pv = ps_big.tile([128, 512], f32, tag="big", name="big")
                pbt = ps_ss.tile([128, 8], f32, tag="ss", name="ss")
                for f in range(4):
                    nc.tensor.matmul(pq[:], hF[:, ts(f, 128)],
                                     swt[f][:, 0:512],
                                     start=(f == 0), stop=(f == 3))
                for f in range(4):
                    nc.tensor.matmul(pk[:], hF[:, ts(f, 128)],
                                     swt[f][:, 512:1024],
                                     start=(f == 0), stop=(f == 3))
                for f in range(4):
                    nc.tensor.matmul(pv[:], hF[:, ts(f, 128)],
                                     swt[f][:, 1024:1536],
                                     start=(f == 0), stop=(f == 3))
                for f in range(4):
                    nc.tensor.matmul(pbt[:], hF[:, ts(f, 128)],
                                     swt[f][:, 1536:1544],
                                     start=(f == 0), stop=(f == 3))

                # softmax(q), softmax(k), sigmoid(beta), kb = beta*k
                stat = ap.tile([128, 40], f32, tag="smstat", name="smstat")
                q_sm = ap.tile([128, 512], bft, tag="q_sm", name="q_sm")
                k_sm = ap.tile([128, 512], bft, tag="k_sm", name="k_sm")
                kb_sm = ap.tile([128, 512], bft, tag="kb_sm", name="kb_sm")
                v_sb = ap.tile([128, 512], bft, tag="v_sb", name="v_sb")
                R = ap.tile([128, 1024], bft, tag="R", name="R")
                bet = stat[:, 16:24]
                nc.scalar.activation(bet, pbt[:], act.Sigmoid)
                for h in range(8):
                    nc.scalar.activation(q_sm[:, ts(h, 64)], pq[:, ts(h, 64)],
                                         act.Exp, accum_out=stat[:, h:h + 1])
                    nc.scalar.activation(k_sm[:, ts(h, 64)], pk[:, ts(h, 64)],
                                         act.Exp,
                                         accum_out=stat[:, 8 + h:9 + h])
                nc.vector.reciprocal(stat[:, 24:32], stat[:, 0:8])   # 1/sum q
                nc.vector.reciprocal(stat[:, 32:40], stat[:, 8:16])  # 1/sum k
                nc.vector.tensor_mul(stat[:, 8:16], bet, stat[:, 32:40])  # b/s
                nc.scalar.copy(v_sb[:], pv[:, 0:512])
                for h in range(8):
                    nc.gpsimd.tensor_scalar_mul(q_sm[:, ts(h, 64)],
                                                q_sm[:, ts(h, 64)],
                                                stat[:, 24 + h:25 + h])
                    nc.gpsimd.tensor_scalar_mul(kb_sm[:, ts(h, 64)],
                                                k_sm[:, ts(h, 64)],
                                                stat[:, 8 + h:9 + h])
                    nc.gpsimd.tensor_scalar_mul(k_sm[:, ts(h, 64)],
                                                k_sm[:, ts(h, 64)],
                                                stat[:, 32 + h:33 + h])
                    # R per head: even [kb | bv], odd [bv | kb]
                    vcol = h * 128 + (64 if h % 2 == 0 else 0)
                    kcol = h * 128 + (0 if h % 2 == 0 else 64)
                    nc.vector.tensor_scalar_mul(R[:, vcol:vcol + 64],
                                                v_sb[:, ts(h, 64)],
                                                bet[:, h:h + 1])
                    nc.gpsimd.tensor_copy(R[:, kcol:kcol + 64],
                                          kb_sm[:, ts(h, 64)])

                if dbg and li == 0 and c == 0:
                    nc.gpsimd.dma_start(dbg_q[:], q_sm[:])
                    nc.gpsimd.dma_start(dbg_k[:], k_sm[:])
                    nc.gpsimd.dma_start(dbg_kb[:], kb_sm[:])
                    nc.gpsimd.dma_start(dbg_v[:], v_sb[:])
                    nc.sync.dma_start(dbg_bet[:], bet)
                # FM transposes: head pair p -> partitions (h%2)*64
                qF = ap.tile([128, 512], bft, tag="qF", name="qF")
                kF = ap.tile([128, 512], bft, tag="kF", name="kF")
                kbF = ap.tile([128, 512], bft, tag="kbF", name="kbF")
                transpose4([q_sm[:, ts(p, 128)] for p in range(4)], qF)
                transpose4([k_sm[:, ts(p, 128)] for p in range(4)], kF)
                transpose4([kb_sm[:, ts(p, 128)] for p in range(4)], kbF)

                for h in range(8):
                    bh = (h % 2) * 64          # FM base partition
                    oh = 64 - bh               # the other half
                    p = h // 2
                    qFh = qF[bh:bh + 64, ts(p, 128)]
                    kFh = kF[bh:bh + 64, ts(p, 128)]
                    kbFh = kbF[bh:bh + 64, ts(p, 128)]
                    kTM = k_sm[:, ts(h, 64)]
                    Rh = R[:, ts(h, 128)]
                    tkc, tvc = (0, 64) if h % 2 == 0 else (64, 0)

                    pss = ps_ss.tile([128, 384], f32, tag="ss", name="ss")
                    nc.tensor.matmul(pss[:, 0:128], kbFh, kFh)      # Sb
                    nc.tensor.matmul(pss[:, 128:256], kFh, kbFh)    # Sb^T
                    nc.tensor.matmul(pss[:, 256:384], kFh, qFh)     # (QK^T)^T
                    A = ip.tile([128, 128], bft, tag="A", name="A")
                    At = ip.tile([128, 128], bft, tag="At", name="At")
                    Mt = ip.tile([128, 128], bft, tag="Mt", name="Mt")
                    nc.vector.tensor_mul(A[:], pss[:, 0:128], mlo[:])
                    nc.vector.tensor_mul(At[:], pss[:, 128:256], mup[:])
                    nc.vector.tensor_mul(Mt[:], pss[:, 256:384], mui[:])

                    wk = ps_wrk.tile([128, 512], f32, tag="wrk", name="wrk")
                    # Neumann order 4: X = (I - A)(I + A^2) R
                    nc.tensor.matmul(wk[:, 0:128], A[:], At[:])     # (A^2)^T
                    A2t = ip.tile([128, 128], bft, tag="A2t", name="A2t")
                    nc.scalar.copy(A2t[:], wk[:, 0:128])
                    Y = ip.tile([128, 128], bft, tag="Y", name="Y")
                    nc.tensor.matmul(wk[:, 128:256], A2t[:], Rh)    # A^2 R
                    nc.vector.tensor_add(Y[:], wk[:, 128:256], Rh)
                    X = ip.tile([128, 128], bft, tag="X", name="X")
                    nc.tensor.matmul(wk[:, 256:384], At[:], Y[:])   # A Y
                    nc.vector.tensor_sub(X[:], Y[:], wk[:, 256:384])

                    # M X (transposed): (M Tk)^T -> cols 0:128 (A2t's cols,
                    # long consumed); (M Tv)^T + O^T accum -> cols 384:512.
                    # Disjoint column ranges keep each accumulation group's
                    # columns private until it closes.
                    nc.tensor.matmul(wk[tkc:tkc + 64, 0:128],
                                     X[:, tkc:tkc + 64], Mt[:])
                    P = ip.tile([128, 128], bft, tag="P", name="P")
                    nc.vector.tensor_sub(P[bh:bh + 64, :], qFh,
                                         wk[bh:bh + 64, 0:128])
                    # O^T = W0 P^T + (M Tv)^T; the group opens and closes on
                    # two adjacent PE ops so no other read of this bank can
                    # land inside it.
                    if c > 0:
                        nc.tensor.matmul(wk[oh:oh + 64, 384:512],
                                         wtb[h][bh:bh + 64, :],
                                         P[bh:bh + 64, :],
                                         start=True, stop=False)
                    nc.tensor.matmul(wk[tvc:tvc + 64, 384:512],
                                     X[:, tvc:tvc + 64], Mt[:],
                                     start=(c == 0), stop=True)
                    nc.scalar.copy(attn[p][oh:oh + 64, ts(c, 128)],
                                   wk[oh:oh + 64, 384:512])

                    if c < NT - 1:
                        # W recurrence (transposed carry Wt [e,d]):
                        # Wt' = Wt + K^T Tv - G'^T Wt,  G' = Tk^T K
                        nGb = ip.tile([128, 64], bft, tag="nG", name="nG")
                        nc.tensor.matmul(wk[bh:bh + 64, 128:192],
                                         X[:, tkc:tkc + 64], kTM)   # G'
                        nc.scalar.mul(nGb[bh:bh + 64, :],
                                      wk[bh:bh + 64, 128:192], -1.0)
                        nc.tensor.matmul(wk[bh:bh + 64, 192:256], kTM,
                                         X[:, tvc:tvc + 64],
                                         start=True, stop=(c == 0))
                        if c > 0:
                            nc.tensor.matmul(wk[bh:bh + 64, 192:256],
                                             nGb[bh:bh + 64, :],
                                             wtb[h][bh:bh + 64, :],
                                             start=False, stop=True)
                        nwtf = sp.tile([128, 64], f32, tag=f"wtf{h}", name=f"wtf{h}")
                        nwtb = sp.tile([128, 64], bft, tag=f"wtb{h}", name=f"wtb{h}")
                        if c > 0:
                            nc.vector.tensor_add(nwtf[bh:bh + 64, :],
                                                 wk[bh:bh + 64, 192:256],
                                                 wtf[h][bh:bh + 64, :])
                        else:
                            nc.vector.tensor_copy(nwtf[bh:bh + 64, :],
                                                  wk[bh:bh + 64, 192:256])
                        nc.scalar.copy(nwtb[bh:bh + 64, :],
                                       nwtf[bh:bh + 64, :])
                        wtf[h] = nwtf
                        wtb[h] = nwtb

                # out proj for this chunk + residual add
                po = ps_big.tile([128, HID], f32, tag="big", name="big")
                for g in range(4):
                    nc.tensor.matmul(po[:], attn[g][:, ts(c, 128)], owt[g][:],
                                     start=(g == 0), stop=(g == 3))
                nc.vector.tensor_add(rt[c][:], rt[c][:], po[:])
            if dbg and li == 0:
                for g in range(4):
                    nc.gpsimd.dma_start(dbg_attn[g], attn[g][:])
                for i2 in range(NT):
                    nc.sync.dma_start(dbg_r1[ts(i2, 128), :], rt[i2][:])

        def ffn():
            BS = min(4, NT)            # token tiles per block
            for blk in range(NT // BS):
                h2F = [ffp.tile([128, BS * 128], bft, tag=f"h2F{f}",
                                name=f"h2F{f}") for f in range(4)]
                h2t = []
                for t in range(BS):
                    i = blk * BS + t
                    h2 = ap.tile([128, HID], bft, tag="h2_tm", name="h2_tm",
                                 bufs=6)
                    layer_norm(i, rt[i], lng[1], lnb[1], h2)
                    h2t.append(h2)
                for f in range(4):
                    transpose4([h2t[t][:, ts(f, 128)] for t in range(BS)],
                               h2F[f])
                rh = [ffp.tile([128, BS * 128], bft, tag=f"rh{ff}",
                               name=f"rh{ff}") for ff in range(16)]
                for ff in range(16):
                    pw = ps_big.tile([128, BS * 128], f32, tag="big",
                                     name="big")
                    for f in range(4):
                        nc.tensor.matmul(pw[:], w1t[f][:, ts(ff, 128)],
                                         h2F[f][:],
                                         start=(f == 0), stop=(f == 3))
                    nc.scalar.activation(rh[ff][:], pw[:], act.Relu,
                                         bias=b1c[ff][:])
                for t in range(BS):
                    i = blk * BS + t
                    pw = ps_big.tile([128, HID], f32, tag="big", name="big")
                    for ff in range(16):
                        nc.tensor.matmul(pw[:], rh[ff][:, ts(t, 128)],
                                         w2t[ff][:],
                                         start=(ff == 0), stop=(ff == 15))
                    nc.vector.tensor_add(rt[i][:], rt[i][:], pw[:])
                    nc.vector.tensor_add(rt[i][:], rt[i][:], b2[:])

        fast_ff(0, 0)
        ffn()
        if dbg:
            for i2 in range(NT):
                nc.sync.dma_start(dbg_r2[ts(i2, 128), :], rt[i2][:])
        fast_ff(1, 2)
        for i in range(NT):
            nc.sync.dma_start(out_d[ts(i, 128), :], rt[i][:])

    nc.compile()
    return nc


def _prep_shared(ip_w, ip_b, fw_ln_g, fw_ln_b, fw_slow_w, fw_out_w,
                 ff_ln_g, ff_ln_b, ff_w1, ff_b1, ff_w2, ff_b2):
    """Host-side preprocessing of weights shared by all cores."""
    ones = np.ones((128, 1), np.float32)

    # slow_w rows permuted to [all q | all k | all v | all beta]
    def perm_slow(w):
        w = w.reshape(H, 3 * D + 1, HID)
        q = w[:, 0:D].reshape(H * D, HID)
        k = w[:, D:2 * D].reshape(H * D, HID)
        v = w[:, 2 * D:3 * D].reshape(H * D, HID)
        b = w[:, 3 * D].reshape(H, HID)
        return np.concatenate([q, k, v, b], 0).T.copy()     # [HID, 1544]

    swT = np.stack([perm_slow(fw_slow_w[0]), perm_slow(fw_slow_w[1])])

    # out_w cols permuted: attn row (g,half,d) -> head h = 2g + (1-half//64)
    hdmap = np.empty(HID, np.int64)
    for row in range(HID):
        g, rr = row // 128, row % 128
        half, d = rr // 64, rr % 64
        h = 2 * g + (1 - half)
        hdmap[row] = h * 64 + d
    owT = np.stack([fw_out_w[0].T[hdmap].copy(), fw_out_w[1].T[hdmap].copy()])

    tri = np.tril(np.ones((128, 128), np.float32), -1)
    masks = np.stack([tri, tri.T, np.triu(np.ones((128, 128), np.float32))])

    return {
        "ipwT": ip_w.T.astype(bfnp).copy(),
        "ipb_bc": (ones * ip_b[None, :]).astype(np.float32),
        "swT": swT.astype(bfnp),
        "owT": owT.astype(bfnp),
        "w1T": ff_w1.T.astype(bfnp).copy(),
        "b1c": ff_b1.reshape(16, 128, 1).astype(np.float32),
        "w2T": ff_w2.T.astype(bfnp).copy(),
        "b2_bc": (ones * ff_b2[None, :]).astype(np.float32),
        "lng_bc": np.stack([ones * fw_ln_g[0], ones * ff_ln_g,
                            ones * fw_ln_g[1]]).astype(np.float32),
        "lnb_bc": np.stack([ones * fw_ln_b[0], ones * ff_ln_b,
                            ones * fw_ln_b[1]]).astype(np.float32),
        "masks": masks,
        "ident": np.eye(128).astype(bfnp),
    }


def kernel(x, ip_w, ip_b, fw_ln_g, fw_ln_b, fw_slow_w, fw_out_w,
           ff_ln_g, ff_ln_b, ff_w1, ff_b1, ff_w2, ff_b2):
    import concourse.bacc as bacc
    from concourse.bass_utils import run_bass_kernel_spmd

    if "nc" not in _CACHED:
        _CACHED["nc"] = build(bacc.Bacc)
    nc = _CACHED["nc"]

    shared = _prep_shared(ip_w, ip_b, fw_ln_g, fw_ln_b, fw_slow_w, fw_out_w,
                          ff_ln_g, ff_ln_b, ff_w1, ff_b1, ff_w2, ff_b2)
    in_maps = []
    for b in range(N_CORES):
        m = dict(shared)
        m["xT"] = x[b].T.astype(bfnp).copy()
        in_maps.append(m)

    res = run_bass_kernel_spmd(nc, in_maps, list(range(N_CORES)))
    out = np.stack([res.results[b]["out"] for b in range(N_CORES)])
    return out.astype(np.float32)


# revision 24
# speedup vs baseline: 1.8001x; 1.8001x over previous
"""DeltaNet model kernel for Trainium2, 8-core data-parallel over batch.

Model (per batch b): input-proj -> fast_ff(0) -> ffn -> fast_ff(1), where
fast_ff = LN -> qkvb proj -> softmax(q,k), sigmoid(beta) -> delta-rule
attention -> out proj -> residual.

The delta-rule scan (W_t = W_{t-1} + beta_t (v_t - W_{t-1} k_t) k_t^T) is
computed chunkwise (C=128) via the UT/WY transform:
  A   = strict_tril(diag(beta) K K^T)            [C,C]
  X   = (I+A)^{-1} [beta*V | beta*K]             (truncated Neumann, order 4:
                                                  X = (I-A)(I+A^2) R)
  M   = tril_incl(Q K^T)
  O_c = (Q - M Tk) W0^T + M Tv
  W1  = W0 + Tv^T K - W0 (Tk^T K)
with [Tv|Tk] = X. Only the W recurrence (4 small matmuls/chunk/head) is
sequential; everything else is chunk-parallel. All matmuls in bf16 with fp32
PSUM accumulation; the residual stream and LN/softmax statistics stay fp32.

Layouts: token-major (TM) = [token partitions, feature free];
feature-major (FM) = [feature partitions, token free]. Per-head parity
bh = (h%2)*64 places head h's FM slices at partition base bh (PE matmuls
require lhsT/rhs on the same base partition).
"""
import numpy as np
import ml_dtypes
from contextlib import ExitStack

bfnp = ml_dtypes.bfloat16

# model dims
S, HID, H, D, FF = 2048, 512, 8, 64, 2048
C = 128               # delta chunk == token tile
EPS = 1e-5
N_CORES = 8
NEUMANN8 = False      # order-8 solve instead of order-4

_CACHED = {}


def build(nc_cls, s=S, n_cores=N_CORES, dbg=False):
    import concourse.bass as bass
    import concourse.tile as tile
    from concourse import mybir

    f32 = mybir.dt.float32
    bft = mybir.dt.bfloat16
    ts = bass.ts
    NT = s // 128          # token tiles / chunks
    act = mybir.ActivationFunctionType
    alu = mybir.AluOpType

    nc = nc_cls("TRN2", target_bir_lowering=False, debug=False,
                num_devices=n_cores)

    # ---- dram I/O ----
    xT_d = nc.dram_tensor("xT", [HID, s], bft, kind="ExternalInput")
    ipwT_d = nc.dram_tensor("ipwT", [HID, HID], bft, kind="ExternalInput")
    ipb_d = nc.dram_tensor("ipb_bc", [128, HID], f32, kind="ExternalInput")
    swT_d = nc.dram_tensor("swT", [2, HID, 1544], bft, kind="ExternalInput")
    owT_d = nc.dram_tensor("owT", [2, HID, HID], bft, kind="ExternalInput")
    w1T_d = nc.dram_tensor("w1T", [HID, FF], bft, kind="ExternalInput")
    b1_d = nc.dram_tensor("b1c", [16, 128, 1], f32, kind="ExternalInput")
    w2T_d = nc.dram_tensor("w2T", [FF, HID], bft, kind="ExternalInput")
    b2_d = nc.dram_tensor("b2_bc", [128, HID], f32, kind="ExternalInput")
    # LN params broadcast to [128, HID]; order: fw0, ff, fw1
    # LN gamma/beta as per-feature columns: [3, 128(feat within tile), 4(tile)]
    lng_d = nc.dram_tensor("lng_bc", [3, 128, 4], f32, kind="ExternalInput")
    lnb_d = nc.dram_tensor("lnb_bc", [3, 128, 4], f32, kind="ExternalInput")
    # masks: 0 = strict lower, 1 = strict upper, 2 = incl upper   [128,128] f32
    msk_d = nc.dram_tensor("masks", [3, 128, 128], f32, kind="ExternalInput")
    idn_d = nc.dram_tensor("ident", [128, 128], bft, kind="ExternalInput")
    out_d = nc.dram_tensor("out", [s, HID], f32, kind="ExternalOutput")
    if dbg:
        dbg_r0 = nc.dram_tensor("dbg_r0", [s, HID], f32, kind="ExternalOutput")
        dbg_h = nc.dram_tensor("dbg_h", [s, HID], f32, kind="ExternalOutput")
        dbg_q = nc.dram_tensor("dbg_q", [128, 512], f32, kind="ExternalOutput")
        dbg_k = nc.dram_tensor("dbg_k", [128, 512], f32, kind="ExternalOutput")
        dbg_kb = nc.dram_tensor("dbg_kb", [128, 512], f32, kind="ExternalOutput")
        dbg_v = nc.dram_tensor("dbg_v", [128, 512], f32, kind="ExternalOutput")
        dbg_bet = nc.dram_tensor("dbg_bet", [128, 8], f32, kind="ExternalOutput")
        dbg_attn = nc.dram_tensor("dbg_attn", [4, 128, s], f32, kind="ExternalOutput")
        dbg_r1 = nc.dram_tensor("dbg_r1", [s, HID], f32, kind="ExternalOutput")
        dbg_r2 = nc.dram_tensor("dbg_r2", [s, HID], f32, kind="ExternalOutput")

    with tile.TileContext(nc) as tc, ExitStack() as ctx:
        wp = ctx.enter_context(tc.tile_pool(name="wts", bufs=1))
        cp = ctx.enter_context(tc.tile_pool(name="const", bufs=1))
        rp = ctx.enter_context(tc.tile_pool(name="res", bufs=1))
        ap = ctx.enter_context(tc.tile_pool(name="acts", bufs=2))
        ip = ctx.enter_context(tc.tile_pool(name="inst", bufs=3))
        sp = ctx.enter_context(tc.tile_pool(name="state", bufs=2))
        anp = ctx.enter_context(tc.tile_pool(name="attn", bufs=1))
        ffp = ctx.enter_context(tc.tile_pool(name="ffn", bufs=1))
        ps_big = ctx.enter_context(
            tc.tile_pool(name="psbig", bufs=2, space=bass.MemorySpace.PSUM))
        ps_wrk = ctx.enter_context(
            tc.tile_pool(name="pswrk", bufs=3, space=bass.MemorySpace.PSUM))
        ps_ss = ctx.enter_context(
            tc.tile_pool(name="psss", bufs=2, space=bass.MemorySpace.PSUM))
        ps_tp = ctx.enter_context(
            tc.tile_pool(name="pstp", bufs=1, space=bass.MemorySpace.PSUM))

        # ---- consts ----
        ident = cp.tile([128, 128], bft, tag="ident", name="ident")
        nc.sync.dma_start(ident[:], idn_d[:])
        mall = cp.tile([128, 384], f32, tag="mall", name="mall")
        for i in range(3):
            nc.sync.dma_start(mall[:, ts(i, 128)], msk_d[i])
        lng = [cp.tile([128, 4], f32, tag=f"lng{i}", name=f"lng{i}") for i in range(3)]
        lnb = [cp.tile([128, 4], f32, tag=f"lnb{i}", name=f"lnb{i}") for i in range(3)]
        for i in range(3):
            nc.sync.dma_start(lng[i][:], lng_d[i])
            nc.sync.dma_start(lnb[i][:], lnb_d[i])
        ipb = cp.tile([128, HID], f32, tag="ipb", name="ipb")
        nc.sync.dma_start(ipb[:], ipb_d[:])
        b2 = cp.tile([128, HID], f32, tag="b2", name="b2")
        nc.sync.dma_start(b2[:], b2_d[:])
        b1c = [cp.tile([128, 1], f32, tag=f"b1c{i}", name=f"b1c{i}") for i in range(16)]
        for i in range(16):
            nc.sync.dma_start(b1c[i][:], b1_d[i])
        epsc = cp.tile([128, 1], f32, tag="epsc", name="epsc")
        nc.gpsimd.memset(epsc[:], EPS)

        # ---- weights ----
        xTt = [wp.tile([128, s], bft, tag=f"xT{f}", name=f"xT{f}") for f in range(4)]
        ipw = [wp.tile([128, HID], bft, tag=f"ipw{f}", name=f"ipw{f}") for f in range(4)]
        for f in range(4):
            nc.sync.dma_start(xTt[f][:], xT_d[ts(f, 128), :])
            nc.sync.dma_start(ipw[f][:], ipwT_d[ts(f, 128), :])
        swt = [wp.tile([128, 1544], bft, tag=f"swt{f}", name=f"swt{f}") for f in range(4)]
        owt = [wp.tile([128, HID], bft, tag=f"owt{f}", name=f"owt{f}") for f in range(4)]
        w1t = [wp.tile([128, FF], bft, tag=f"w1t{f}", name=f"w1t{f}") for f in range(4)]
        w2t = [wp.tile([128, HID], bft, tag=f"w2t{f}", name=f"w2t{f}") for f in range(16)]
        for f in range(4):
            nc.sync.dma_start(swt[f][:], swT_d[0, ts(f, 128), :])
            nc.sync.dma_start(owt[f][:], owT_d[0, ts(f, 128), :])
            nc.sync.dma_start(w1t[f][:], w1T_d[ts(f, 128), :])
        for f in range(16):
            nc.sync.dma_start(w2t[f][:], w2T_d[ts(f, 128), :])

        # ---- residual: input proj ----
        rt = [rp.tile([128, HID], f32, tag=f"r{i}", name=f"r{i}") for i in range(NT)]
        for i in range(NT):
            pb = ps_big.tile([128, HID], f32, tag="big", name="big")
            for f in range(4):
                nc.tensor.matmul(pb[:], xTt[f][:, ts(i, 128)], ipw[f][:],
                                 start=(f == 0), stop=(f == 3))
            nc.vector.tensor_add(rt[i][:], pb[:], ipb[:])
            if dbg:
                nc.sync.dma_start(dbg_r0[ts(i, 128), :], rt[i][:])

        def ln_stats(tiles):
            """Batched LN stats for a group of token tiles.
            Returns (m, rstd) [128, len(tiles)]; one ACT Sqrt per group."""
            n = len(tiles)
            st = ap.tile([128, 4 * 8], f32, tag="lnstat", name="lnstat",
                         bufs=3)
            scr = ap.tile([128, HID], f32, tag="lnscr", name="lnscr")
            sm, sq = st[:, 0:n], st[:, 8:8 + n]
            m, m2 = st[:, 16:16 + n], st[:, 24:24 + n]
            for j, r_tile in enumerate(tiles):
                nc.vector.tensor_reduce(sm[:, j:j + 1], r_tile[:],
                                        mybir.AxisListType.X, alu.add)
                nc.scalar.activation(scr[:], r_tile[:], act.Square,
                                     accum_out=sq[:, j:j + 1])
            nc.scalar.mul(m, sm, 1.0 / HID)
            nc.scalar.mul(sq, sq, 1.0 / HID)
            nc.vector.tensor_tensor(m2, m, m, alu.mult)
            nc.vector.tensor_sub(sq, sq, m2)                      # var
            nc.scalar.activation(sq, sq, act.Sqrt, bias=epsc[:])
            nc.vector.reciprocal(sq, sq)                          # rstd
            return st

        def ln_apply(st, j, r_tile, h_out):
            nc.vector.tensor_scalar(h_out[:], r_tile[:], st[:, 16 + j:17 + j],
                                    st[:, 8 + j:9 + j], alu.subtract, alu.mult)

        def transpose4(src_aps, dst_tile, gb=None, gcol=None):
            """PE-transpose up to four [128,128] bf16 APs into dst_tile.
            gb=(g, b, [cols]) applies per-partition gamma/beta per block;
            gcol=(g, b, f) applies one gamma/beta to the whole copy."""
            tp = ps_tp.tile([128, 512], bft, tag="tp", name="tp")
            for j, a in enumerate(src_aps):
                nc.tensor.transpose(tp[:, ts(j, 128)], a, ident[:])
            n = len(src_aps)
            if gb is not None:
                g, b = gb
                for j in range(n):
                    nc.vector.tensor_scalar(dst_tile[:, ts(j, 128)],
                                            tp[:, ts(j, 128)],
                                            g[:, j:j + 1], b[:, j:j + 1],
                                            alu.mult, alu.add)
            elif gcol is not None:
                g, b, f = gcol
                nc.vector.tensor_scalar(dst_tile[:], tp[:, 0:n * 128],
                                        g[:, f:f + 1], b[:, f:f + 1],
                                        alu.mult, alu.add)
            else:
                nc.scalar.copy(dst_tile[:], tp[:, 0:n * 128])

        def fast_ff(li, lnidx):
            attn = [anp.tile([128, s], bft, tag=f"attn{g}", name=f"attn{g}") for g in range(4)]
            if li > 0:
                for f in range(4):
                    nc.sync.dma_start(swt[f][:], swT_d[li, ts(f, 128), :])
                    nc.sync.dma_start(owt[f][:], owT_d[li, ts(f, 128), :])
            wtf = {}
            wtb = {}
            sts = {}
            for c in range(NT):
                # batched LN stats every 4 chunks
                if c % 4 == 0:
                    sts[c // 4] = ln_stats([rt[i] for i in
                                            range(c, min(c + 4, NT))])
                h_tm = ap.tile([128, HID], bft, tag="h_tm", name="h_tm")
                ln_apply(sts[c // 4], c % 4, rt[c], h_tm)
                if dbg and li == 0:
                    nc.gpsimd.dma_start(dbg_h[ts(c, 128), :], h_tm[:])
                hF = ap.tile([128, HID], bft, tag="hF", name="hF")
                transpose4([h_tm[:, ts(f, 128)] for f in range(4)], hF,
                           gb=(lng[lnidx], lnb[lnidx]))

                # qkvb projection (cols: q 0:512, k 512:1024, v 1024:1536,
                # beta 1536:1544 -- host permuted)
                pq = ps_big.tile([128, 512], f32, tag="big", name="big")
                pk = ps_big.tile([128, 512], f32, tag="big", name="big")
                pv = ps_big.tile([128, 512], f32, tag="big", name="big")
                pbt = ps_ss.tile([128, 8], f32, tag="ss", name="ss")
                for f in range(4):
                    nc.tensor.matmul(pq[:], hF[:, ts(f, 128)],
                                     swt[f][:, 0:512],
                                     start=(f == 0), stop=(f == 3))
                for f in range(4):
                    nc.tensor.matmul(pk[:], hF[:, ts(f, 128)],
                                     swt[f][:, 512:1024],
                                     start=(f == 0), stop=(f == 3))
                for f in range(4):
                    nc.tensor.matmul(pv[:], hF[:, ts(f, 128)],
                                     swt[f][:, 1024:1536],
                                     start=(f == 0), stop=(f == 3))
                for f in range(4):
                    nc.tensor.matmul(pbt[:], hF[:, ts(f, 128)],
                                     swt[f][:, 1536:1544],
                                     start=(f == 0), stop=(f == 3))

                # softmax(q), softmax(k); sigmoid(beta) via Exp (keeps the
                # ACT table on Exp); kb = beta*k; R built per head
                stat = ap.tile([128, 40], f32, tag="smstat", name="smstat")
                q_exp = ap.tile([128, 512], bft, tag="q_exp", name="q_exp")
                k_exp = ap.tile([128, 512], bft, tag="k_exp", name="k_exp")
                q_sm = ap.tile([128, 512], bft, tag="q_sm", name="q_sm")
                k_sm = ap.tile([128, 512], bft, tag="k_sm", name="k_sm")
                kb_sm = ap.tile([128, 512], bft, tag="kb_sm", name="kb_sm")
                R = ap.tile([128, 1024], bft, tag="R", name="R")
                bet = stat[:, 16:24]
                nc.scalar.activation(bet, pbt[:], act.Exp, scale=-1.0)
                nc.vector.tensor_scalar_add(bet, bet, 1.0)
                nc.vector.reciprocal(bet, bet)                    # sigmoid
                nc.scalar.activation(q_exp[:], pq[:], act.Exp)
                nc.scalar.activation(k_exp[:], pk[:], act.Exp)
                nc.vector.tensor_reduce(
                    stat[:, 0:8], q_exp[:].rearrange("p (h d) -> p h d", h=8),
                    mybir.AxisListType.X, alu.add)
                nc.vector.tensor_reduce(
                    stat[:, 8:16], k_exp[:].rearrange("p (h d) -> p h d", h=8),
                    mybir.AxisListType.X, alu.add)
                nc.vector.reciprocal(stat[:, 24:32], stat[:, 0:8])   # 1/sum q
                nc.vector.reciprocal(stat[:, 32:40], stat[:, 8:16])  # 1/sum k
                nc.vector.tensor_mul(stat[:, 8:16], bet, stat[:, 32:40])  # b/s
                for h in range(8):
                    vcol = h * 128 + (64 if h % 2 == 0 else 0)
                    kcol = h * 128 + (0 if h % 2 == 0 else 64)
                    nc.vector.tensor_scalar_mul(q_sm[:, ts(h, 64)],
                                                q_exp[:, ts(h, 64)],
                                                stat[:, 24 + h:25 + h])
                    nc.vector.tensor_scalar_mul(kb_sm[:, ts(h, 64)],
                                                k_exp[:, ts(h, 64)],
                                                stat[:, 8 + h:9 + h])
                    nc.vector.tensor_scalar_mul(k_sm[:, ts(h, 64)],
                                                k_exp[:, ts(h, 64)],
                                                stat[:, 32 + h:33 + h])
                    # beta*v straight from the v PSUM
                    nc.vector.tensor_scalar_mul(R[:, vcol:vcol + 64],
                                                pv[:, ts(h, 64)],
                                                bet[:, h:h + 1])
                    nc.vector.tensor_copy(R[:, kcol:kcol + 64],
                                          kb_sm[:, ts(h, 64)])

                if dbg and li == 0 and c == 0:
                    nc.gpsimd.dma_start(dbg_q[:], q_sm[:])
                    nc.gpsimd.dma_start(dbg_k[:], k_sm[:])
                    nc.gpsimd.dma_start(dbg_kb[:], kb_sm[:])
                    nc.sync.dma_start(dbg_bet[:], bet)
                # FM transposes: head pair p -> partitions (h%2)*64
                qF = ap.tile([128, 512], bft, tag="qF", name="qF")
                kF = ap.tile([128, 512], bft, tag="kF", name="kF")
                kbF = ap.tile([128, 512], bft, tag="kbF", name="kbF")
                transpose4([q_sm[:, ts(p, 128)] for p in range(4)], qF)
                transpose4([k_sm[:, ts(p, 128)] for p in range(4)], kF)
                transpose4([kb_sm[:, ts(p, 128)] for p in range(4)], kbF)

                for h in range(8):
                    bh = (h % 2) * 64          # FM base partition
                    oh = 64 - bh               # the other half
                    p = h // 2
                    qFh = qF[bh:bh + 64, ts(p, 128)]
                    kFh = kF[bh:bh + 64, ts(p, 128)]
                    kbFh = kbF[bh:bh + 64, ts(p, 128)]
                    kTM = k_sm[:, ts(h, 64)]
                    Rh = R[:, ts(h, 128)]
                    tkc, tvc = (0, 64) if h % 2 == 0 else (64, 0)

                    pss = ps_ss.tile([128, 384], f32, tag="ss", name="ss")
                    nc.tensor.matmul(pss[:, 0:128], kbFh, kFh)      # Sb
                    nc.tensor.matmul(pss[:, 128:256], kFh, kbFh)    # Sb^T
                    nc.tensor.matmul(pss[:, 256:384], kFh, qFh)     # (QK^T)^T
                    AAM = ip.tile([128, 384], bft, tag="AAM", name="AAM")
                    nc.vector.tensor_mul(AAM[:], pss[:, 0:384], mall[:])
                    A = AAM[:, 0:128]
                    At = AAM[:, 128:256]
                    Mt = AAM[:, 256:384]

                    wk = ps_wrk.tile([128, 512], f32, tag="wrk", name="wrk")
                    # Neumann order 4: X = (I - A)(I + A^2) R
                    nc.tensor.matmul(wk[:, 0:128], A, At)           # (A^2)^T
                    A2t = ip.tile([128, 128], bft, tag="A2t", name="A2t")
                    nc.scalar.copy(A2t[:], wk[:, 0:128])
                    Y = ip.tile([128, 128], bft, tag="Y", name="Y")
                    nc.tensor.matmul(wk[:, 128:256], A2t[:], Rh)    # A^2 R
                    nc.vector.tensor_add(Y[:], wk[:, 128:256], Rh)
                    X = ip.tile([128, 128], bft, tag="X", name="X")
                    nc.tensor.matmul(wk[:, 256:384], At, Y[:])      # A Y
                    nc.vector.tensor_sub(X[:], Y[:], wk[:, 256:384])

                    # M X (transposed): (M Tk)^T -> cols 0:128 (A2t's cols,
                    # long consumed); (M Tv)^T + O^T accum -> cols 384:512.
                    # Disjoint column ranges keep each accumulation group's
                    # columns private until it closes.
                    nc.tensor.matmul(wk[tkc:tkc + 64, 0:128],
                                     X[:, tkc:tkc + 64], Mt)
                    P = ip.tile([128, 128], bft, tag="P", name="P")
                    nc.vector.tensor_sub(P[bh:bh + 64, :], qFh,
                                         wk[bh:bh + 64, 0:128])
                    # O^T = W0 P^T + (M Tv)^T; the group opens and closes on
                    # two adjacent PE ops so no other read of this bank can
                    # land inside it.
                    if c > 0:
                        nc.tensor.matmul(wk[oh:oh + 64, 384:512],
                                         wtb[h][bh:bh + 64, :],
                                         P[bh:bh + 64, :],
                                         start=True, stop=False)
                    nc.tensor.matmul(wk[tvc:tvc + 64, 384:512],
                                     X[:, tvc:tvc + 64], Mt,
                                     start=(c == 0), stop=True)
                    nc.scalar.copy(attn[p][oh:oh + 64, ts(c, 128)],
                                   wk[oh:oh + 64, 384:512])

                    if c < NT - 1:
                        # W recurrence (transposed carry Wt [e,d]):
                        # Wt' = Wt + K^T Tv - G'^T Wt,  G' = Tk^T K
                        nGb = ip.tile([128, 64], bft, tag="nG", name="nG")
                        nc.tensor.matmul(wk[bh:bh + 64, 128:192],
                                         X[:, tkc:tkc + 64], kTM)   # G'
                        nc.scalar.mul(nGb[bh:bh + 64, :],
                                      wk[bh:bh + 64, 128:192], -1.0)
                        nc.tensor.matmul(wk[bh:bh + 64, 192:256], kTM,
                                         X[:, tvc:tvc + 64],
                                         start=True, stop=(c == 0))
                        if c > 0:
                            nc.tensor.matmul(wk[bh:bh + 64, 192:256],
                                             nGb[bh:bh + 64, :],
                                             wtb[h][bh:bh + 64, :],
                                             start=False, stop=True)
                        nwtf = sp.tile([128, 64], f32, tag=f"wtf{h}", name=f"wtf{h}")
                        nwtb = sp.tile([128, 64], bft, tag=f"wtb{h}", name=f"wtb{h}")
                        if c > 0:
                            nc.vector.tensor_add(nwtf[bh:bh + 64, :],
                                                 wk[bh:bh + 64, 192:256],
                                                 wtf[h][bh:bh + 64, :])
                        else:
                            nc.vector.tensor_copy(nwtf[bh:bh + 64, :],
                                                  wk[bh:bh + 64, 192:256])
                        nc.scalar.copy(nwtb[bh:bh + 64, :],
                                       nwtf[bh:bh + 64, :])
                        wtf[h] = nwtf
                        wtb[h] = nwtb

                # out proj for this chunk + residual add
                po = ps_big.tile([128, HID], f32, tag="big", name="big")
                for g in range(4):
                    nc.tensor.matmul(po[:], attn[g][:, ts(c, 128)], owt[g][:],
                                     start=(g == 0), stop=(g == 3))
                nc.vector.tensor_add(rt[c][:], rt[c][:], po[:])
            if dbg and li == 0:
                for g in range(4):
                    nc.gpsimd.dma_start(dbg_attn[g], attn[g][:])
                for i2 in range(NT):
                    nc.sync.dma_start(dbg_r1[ts(i2, 128), :], rt[i2][:])

        def ffn():
            BS = min(4, NT)            # token tiles per block
            for blk in range(NT // BS):
                h2F = [ffp.tile([128, BS * 128], bft, tag=f"h2F{f}",
                                name=f"h2F{f}") for f in range(4)]
                stf = ln_stats([rt[blk * BS + t] for t in range(BS)])
                h2t = []
                for t in range(BS):
                    i = blk * BS + t
                    h2 = ap.tile([128, HID], bft, tag="h2_tm", name="h2_tm",
                                 bufs=6)
                    ln_apply(stf, t, rt[i], h2)
                    h2t.append(h2)
                for f in range(4):
                    transpose4([h2t[t][:, ts(f, 128)] for t in range(BS)],
                               h2F[f], gcol=(lng[1], lnb[1], f))
                rh = [ffp.tile([128, BS * 128], bft, tag=f"rh{ff}",
                               name=f"rh{ff}") for ff in range(16)]
                for ff in range(16):
                    pw = ps_big.tile([128, BS * 128], f32, tag="big",
                                     name="big")
                    for f in range(4):
                        nc.tensor.matmul(pw[:], w1t[f][:, ts(ff, 128)],
                                         h2F[f][:],
                                         start=(f == 0), stop=(f == 3))
                    nc.scalar.activation(rh[ff][:], pw[:], act.Relu,
                                         bias=b1c[ff][:])
                for t in range(BS):
                    i = blk * BS + t
                    pw = ps_big.tile([128, HID], f32, tag="big", name="big")
                    for ff in range(16):
                        nc.tensor.matmul(pw[:], rh[ff][:, ts(t, 128)],
                                         w2t[ff][:],
                                         start=(ff == 0), stop=(ff == 15))
                    nc.vector.tensor_add(rt[i][:], rt[i][:], pw[:])
                    nc.vector.tensor_add(rt[i][:], rt[i][:], b2[:])

        fast_ff(0, 0)
        ffn()
        if dbg:
            for i2 in range(NT):
                nc.sync.dma_start(dbg_r2[ts(i2, 128), :], rt[i2][:])
        fast_ff(1, 2)
        for i in range(NT):
            nc.sync.dma_start(out_d[ts(i, 128), :], rt[i][:])

    nc.compile()
    return nc


def _prep_shared(ip_w, ip_b, fw_ln_g, fw_ln_b, fw_slow_w, fw_out_w,
                 ff_ln_g, ff_ln_b, ff_w1, ff_b1, ff_w2, ff_b2):
    """Host-side preprocessing of weights shared by all cores."""
    ones = np.ones((128, 1), np.float32)

    # slow_w rows permuted to [all q | all k | all v | all beta]
    def perm_slow(w):
        w = w.reshape(H, 3 * D + 1, HID)
        q = w[:, 0:D].reshape(H * D, HID)
        k = w[:, D:2 * D].reshape(H * D, HID)
        v = w[:, 2 * D:3 * D].reshape(H * D, HID)
        b = w[:, 3 * D].reshape(H, HID)
        return np.concatenate([q, k, v, b], 0).T.copy()     # [HID, 1544]

    swT = np.stack([perm_slow(fw_slow_w[0]), perm_slow(fw_slow_w[1])])

    # out_w cols permuted: attn row (g,half,d) -> head h = 2g + (1-half//64)
    hdmap = np.empty(HID, np.int64)
    for row in range(HID):
        g, rr = row // 128, row % 128
        half, d = rr // 64, rr % 64
        h = 2 * g + (1 - half)
        hdmap[row] = h * 64 + d
    owT = np.stack([fw_out_w[0].T[hdmap].copy(), fw_out_w[1].T[hdmap].copy()])

    tri = np.tril(np.ones((128, 128), np.float32), -1)
    masks = np.stack([tri, tri.T, np.triu(np.ones((128, 128), np.float32))])

    return {
        "ipwT": ip_w.T.astype(bfnp).copy(),
        "ipb_bc": (ones * ip_b[None, :]).astype(np.float32),
        "swT": swT.astype(bfnp),
        "owT": owT.astype(bfnp),
        "w1T": ff_w1.T.astype(bfnp).copy(),
        "b1c": ff_b1.reshape(16, 128, 1).astype(np.float32),
        "w2T": ff_w2.T.astype(bfnp).copy(),
        "b2_bc": (ones * ff_b2[None, :]).astype(np.float32),
        "lng_bc": np.stack([fw_ln_g[0], ff_ln_g, fw_ln_g[1]])
        .reshape(3, 4, 128).transpose(0, 2, 1).astype(np.float32).copy(),
        "lnb_bc": np.stack([fw_ln_b[0], ff_ln_b, fw_ln_b[1]])
        .reshape(3, 4, 128).transpose(0, 2, 1).astype(np.float32).copy(),
        "masks": masks,
        "ident": np.eye(128).astype(bfnp),
    }


def kernel(x, ip_w, ip_b, fw_ln_g, fw_ln_b, fw_slow_w, fw_out_w,
           ff_ln_g, ff_ln_b, ff_w1, ff_b1, ff_w2, ff_b2):
    import concourse.bacc as bacc
    from concourse.bass_utils import run_bass_kernel_spmd

    if "nc" not in _CACHED:
        _CACHED["nc"] = build(bacc.Bacc)
    nc = _CACHED["nc"]

    shared = _prep_shared(ip_w, ip_b, fw_ln_g, fw_ln_b, fw_slow_w, fw_out_w,
                          ff_ln_g, ff_ln_b, ff_w1, ff_b1, ff_w2, ff_b2)
    in_maps = []
    for b in range(N_CORES):
        m = dict(shared)
        m["xT"] = x[b].T.astype(bfnp).copy()
        in_maps.append(m)

    res = run_bass_kernel_spmd(nc, in_maps, list(range(N_CORES)))
    out = np.stack([res.results[b]["out"] for b in range(N_CORES)])
    return out.astype(np.float32)


# revision 26
# speedup vs baseline: 1.9489x; 1.0827x over previous
"""DeltaNet model kernel for Trainium2, 8-core data-parallel over batch.

Model (per batch b): input-proj -> fast_ff(0) -> ffn -> fast_ff(1), where
fast_ff = LN -> qkvb proj -> softmax(q,k), sigmoid(beta) -> delta-rule
attention -> out proj -> residual.

The delta-rule scan (W_t = W_{t-1} + beta_t (v_t - W_{t-1} k_t) k_t^T) is
computed chunkwise (C=128) via the UT/WY transform:
  A   = strict_tril(diag(beta) K K^T)            [C,C]
  X   = (I+A)^{-1} [beta*V | beta*K]             (truncated Neumann, order 4:
                                                  X = (I-A)(I+A^2) R)
  M   = tril_incl(Q K^T)
  O_c = (Q - M Tk) W0^T + M Tv
  W1  = W0 + Tv^T K - W0 (Tk^T K)
with [Tv|Tk] = X. Only the W recurrence (4 small matmuls/chunk/head) is
sequential; everything else is chunk-parallel. All matmuls in bf16 with fp32
PSUM accumulation; the residual stream and LN/softmax statistics stay fp32.

Layouts: token-major (TM) = [token partitions, feature free];
feature-major (FM) = [feature partitions, token free]. Per-head parity
bh = (h%2)*64 places head h's FM slices at partition base bh (PE matmuls
require lhsT/rhs on the same base partition).
"""
import numpy as np
import ml_dtypes
from contextlib import ExitStack

bfnp = ml_dtypes.bfloat16

# model dims
S, HID, H, D, FF = 2048, 512, 8, 64, 2048
C = 128               # delta chunk == token tile
EPS = 1e-5
N_CORES = 8
NEUMANN8 = False      # order-8 solve instead of order-4

_CACHED = {}


def build(nc_cls, s=S, n_cores=N_CORES, dbg=False):
    import concourse.bass as bass
    import concourse.tile as tile
    from concourse import mybir

    f32 = mybir.dt.float32
    bft = mybir.dt.bfloat16
    ts = bass.ts
    NT = s // 128          # token tiles / chunks
    act = mybir.ActivationFunctionType
    alu = mybir.AluOpType

    nc = nc_cls("TRN2", target_bir_lowering=False, debug=False,
                num_devices=n_cores)

    # ---- dram I/O ----
    xT_d = nc.dram_tensor("xT", [HID, s], bft, kind="ExternalInput")
    ipwT_d = nc.dram_tensor("ipwT", [HID, HID], bft, kind="ExternalInput")
    ipb_d = nc.dram_tensor("ipb_bc", [128, HID], f32, kind="ExternalInput")
    swT_d = nc.dram_tensor("swT", [2, HID, 1544], bft, kind="ExternalInput")
    owT_d = nc.dram_tensor("owT", [2, HID, HID], bft, kind="ExternalInput")
    w1T_d = nc.dram_tensor("w1T", [HID, FF], bft, kind="ExternalInput")
    b1_d = nc.dram_tensor("b1c", [16, 128, 1], f32, kind="ExternalInput")
    w2T_d = nc.dram_tensor("w2T", [FF, HID], bft, kind="ExternalInput")
    b2_d = nc.dram_tensor("b2_bc", [128, HID], f32, kind="ExternalInput")
    # LN params broadcast to [128, HID]; order: fw0, ff, fw1
    # LN gamma/beta as per-feature columns: [3, 128(feat within tile), 4(tile)]
    lng_d = nc.dram_tensor("lng_bc", [3, 128, 4], f32, kind="ExternalInput")
    lnb_d = nc.dram_tensor("lnb_bc", [3, 128, 4], f32, kind="ExternalInput")
    # masks: 0 = strict lower, 1 = strict upper, 2 = incl upper   [128,128] f32
    msk_d = nc.dram_tensor("masks", [3, 128, 128], f32, kind="ExternalInput")
    idn_d = nc.dram_tensor("ident", [128, 128], bft, kind="ExternalInput")
    out_d = nc.dram_tensor("out", [s, HID], f32, kind="ExternalOutput")
    if dbg:
        dbg_r0 = nc.dram_tensor("dbg_r0", [s, HID], f32, kind="ExternalOutput")
        dbg_h = nc.dram_tensor("dbg_h", [s, HID], f32, kind="ExternalOutput")
        dbg_q = nc.dram_tensor("dbg_q", [128, 512], f32, kind="ExternalOutput")
        dbg_k = nc.dram_tensor("dbg_k", [128, 512], f32, kind="ExternalOutput")
        dbg_kb = nc.dram_tensor("dbg_kb", [128, 512], f32, kind="ExternalOutput")
        dbg_v = nc.dram_tensor("dbg_v", [128, 512], f32, kind="ExternalOutput")
        dbg_bet = nc.dram_tensor("dbg_bet", [128, 8], f32, kind="ExternalOutput")
        dbg_attn = nc.dram_tensor("dbg_attn", [4, 128, s], f32, kind="ExternalOutput")
        dbg_r1 = nc.dram_tensor("dbg_r1", [s, HID], f32, kind="ExternalOutput")
        dbg_r2 = nc.dram_tensor("dbg_r2", [s, HID], f32, kind="ExternalOutput")

    with tile.TileContext(nc) as tc, ExitStack() as ctx:
        wp = ctx.enter_context(tc.tile_pool(name="wts", bufs=1))
        cp = ctx.enter_context(tc.tile_pool(name="const", bufs=1))
        rp = ctx.enter_context(tc.tile_pool(name="res", bufs=1))
        ap = ctx.enter_context(tc.tile_pool(name="acts", bufs=2))
        ip = ctx.enter_context(tc.tile_pool(name="inst", bufs=3))
        sp = ctx.enter_context(tc.tile_pool(name="state", bufs=2))
        anp = ctx.enter_context(tc.tile_pool(name="attn", bufs=1))
        ffp = ctx.enter_context(tc.tile_pool(name="ffn", bufs=1))
        ps_big = ctx.enter_context(
            tc.tile_pool(name="psbig", bufs=2, space=bass.MemorySpace.PSUM))
        ps_wrk = ctx.enter_context(
            tc.tile_pool(name="pswrk", bufs=3, space=bass.MemorySpace.PSUM))
        ps_ss = ctx.enter_context(
            tc.tile_pool(name="psss", bufs=2, space=bass.MemorySpace.PSUM))
        ps_tp = ctx.enter_context(
            tc.tile_pool(name="pstp", bufs=1, space=bass.MemorySpace.PSUM))

        # ---- consts ----
        ident = cp.tile([128, 128], bft, tag="ident", name="ident")
        nc.sync.dma_start(ident[:], idn_d[:])
        mall = cp.tile([128, 384], f32, tag="mall", name="mall")
        for i in range(3):
            nc.sync.dma_start(mall[:, ts(i, 128)], msk_d[i])
        lng = [cp.tile([128, 4], f32, tag=f"lng{i}", name=f"lng{i}") for i in range(3)]
        lnb = [cp.tile([128, 4], f32, tag=f"lnb{i}", name=f"lnb{i}") for i in range(3)]
        for i in range(3):
            nc.sync.dma_start(lng[i][:], lng_d[i])
            nc.sync.dma_start(lnb[i][:], lnb_d[i])
        ipb = cp.tile([128, HID], f32, tag="ipb", name="ipb")
        nc.sync.dma_start(ipb[:], ipb_d[:])
        b2 = cp.tile([128, HID], f32, tag="b2", name="b2")
        nc.sync.dma_start(b2[:], b2_d[:])
        b1c = [cp.tile([128, 1], f32, tag=f"b1c{i}", name=f"b1c{i}") for i in range(16)]
        for i in range(16):
            nc.sync.dma_start(b1c[i][:], b1_d[i])
        epsc = cp.tile([128, 1], f32, tag="epsc", name="epsc")
        nc.gpsimd.memset(epsc[:], EPS)

        # ---- weights ----
        xTt = [wp.tile([128, s], bft, tag=f"xT{f}", name=f"xT{f}") for f in range(4)]
        ipw = [wp.tile([128, HID], bft, tag=f"ipw{f}", name=f"ipw{f}") for f in range(4)]
        for f in range(4):
            nc.sync.dma_start(xTt[f][:], xT_d[ts(f, 128), :])
            nc.sync.dma_start(ipw[f][:], ipwT_d[ts(f, 128), :])
        swt = [wp.tile([128, 1544], bft, tag=f"swt{f}", name=f"swt{f}") for f in range(4)]
        owt = [wp.tile([128, HID], bft, tag=f"owt{f}", name=f"owt{f}") for f in range(4)]
        w1t = [wp.tile([128, FF], bft, tag=f"w1t{f}", name=f"w1t{f}") for f in range(4)]
        w2t = [wp.tile([128, HID], bft, tag=f"w2t{f}", name=f"w2t{f}") for f in range(16)]
        for f in range(4):
            nc.sync.dma_start(swt[f][:], swT_d[0, ts(f, 128), :])
            nc.sync.dma_start(owt[f][:], owT_d[0, ts(f, 128), :])
            nc.sync.dma_start(w1t[f][:], w1T_d[ts(f, 128), :])
        for f in range(16):
            nc.sync.dma_start(w2t[f][:], w2T_d[ts(f, 128), :])

        # ---- residual: input proj ----
        rt = [rp.tile([128, HID], f32, tag=f"r{i}", name=f"r{i}") for i in range(NT)]
        for i in range(NT):
            pb = ps_big.tile([128, HID], f32, tag="big", name="big")
            for f in range(4):
                nc.tensor.matmul(pb[:], xTt[f][:, ts(i, 128)], ipw[f][:],
                                 start=(f == 0), stop=(f == 3))
            nc.vector.tensor_add(rt[i][:], pb[:], ipb[:])
            if dbg:
                nc.sync.dma_start(dbg_r0[ts(i, 128), :], rt[i][:])

        def ln_stats(tiles):
            """Batched LN stats for a group of token tiles.
            Returns (m, rstd) [128, len(tiles)]; one ACT Sqrt per group."""
            n = len(tiles)
            st = ap.tile([128, 4 * 8], f32, tag="lnstat", name="lnstat",
                         bufs=3)
            scr = ap.tile([128, HID], f32, tag="lnscr", name="lnscr")
            sm, sq = st[:, 0:n], st[:, 8:8 + n]
            m, m2 = st[:, 16:16 + n], st[:, 24:24 + n]
            for j, r_tile in enumerate(tiles):
                nc.vector.tensor_reduce(sm[:, j:j + 1], r_tile[:],
                                        mybir.AxisListType.X, alu.add)
                nc.scalar.activation(scr[:], r_tile[:], act.Square,
                                     accum_out=sq[:, j:j + 1])
            nc.scalar.mul(m, sm, 1.0 / HID)
            nc.scalar.mul(sq, sq, 1.0 / HID)
            nc.vector.tensor_tensor(m2, m, m, alu.mult)
            nc.vector.tensor_sub(sq, sq, m2)                      # var
            nc.scalar.activation(sq, sq, act.Sqrt, bias=epsc[:])
            nc.vector.reciprocal(sq, sq)                          # rstd
            return st

        def ln_apply(st, j, r_tile, h_out):
            nc.vector.tensor_scalar(h_out[:], r_tile[:], st[:, 16 + j:17 + j],
                                    st[:, 8 + j:9 + j], alu.subtract, alu.mult)

        def transpose4(src_aps, dst_tile, gb=None, gcol=None):
            """PE-transpose up to four [128,128] bf16 APs into dst_tile.
            gb=(g, b, [cols]) applies per-partition gamma/beta per block;
            gcol=(g, b, f) applies one gamma/beta to the whole copy."""
            tp = ps_tp.tile([128, 512], bft, tag="tp", name="tp")
            for j, a in enumerate(src_aps):
                nc.tensor.transpose(tp[:, ts(j, 128)], a, ident[:])
            n = len(src_aps)
            if gb is not None:
                g, b = gb
                for j in range(n):
                    nc.vector.tensor_scalar(dst_tile[:, ts(j, 128)],
                                            tp[:, ts(j, 128)],
                                            g[:, j:j + 1], b[:, j:j + 1],
                                            alu.mult, alu.add)
            elif gcol is not None:
                g, b, f = gcol
                nc.vector.tensor_scalar(dst_tile[:], tp[:, 0:n * 128],
                                        g[:, f:f + 1], b[:, f:f + 1],
                                        alu.mult, alu.add)
            else:
                nc.scalar.copy(dst_tile[:], tp[:, 0:n * 128])

        def fast_ff(li, lnidx):
            attn = [anp.tile([128, s], bft, tag=f"attn{g}", name=f"attn{g}") for g in range(4)]
            if li > 0:
                for f in range(4):
                    nc.sync.dma_start(swt[f][:], swT_d[li, ts(f, 128), :])
                    nc.sync.dma_start(owt[f][:], owT_d[li, ts(f, 128), :])
            wtf = {}
            wtb = {}
            sts = {}
            for c in range(NT):
                # batched LN stats every 4 chunks
                if c % 4 == 0:
                    sts[c // 4] = ln_stats([rt[i] for i in
                                            range(c, min(c + 4, NT))])
                h_tm = ap.tile([128, HID], bft, tag="h_tm", name="h_tm")
                ln_apply(sts[c // 4], c % 4, rt[c], h_tm)
                if dbg and li == 0:
                    nc.gpsimd.dma_start(dbg_h[ts(c, 128), :], h_tm[:])
                hF = ap.tile([128, HID], bft, tag="hF", name="hF")
                transpose4([h_tm[:, ts(f, 128)] for f in range(4)], hF,
                           gb=(lng[lnidx], lnb[lnidx]))

                # qkvb projection (cols: q 0:512, k 512:1024, v 1024:1536,
                # beta 1536:1544 -- host permuted)
                pq = ps_big.tile([128, 512], f32, tag="big", name="big")
                pk = ps_big.tile([128, 512], f32, tag="big", name="big")
                pv = ps_big.tile([128, 512], f32, tag="big", name="big")
                pbt = ps_ss.tile([128, 8], f32, tag="ss", name="ss")
                for f in range(4):
                    nc.tensor.matmul(pq[:], hF[:, ts(f, 128)],
                                     swt[f][:, 0:512],
                                     start=(f == 0), stop=(f == 3))
                for f in range(4):
                    nc.tensor.matmul(pk[:], hF[:, ts(f, 128)],
                                     swt[f][:, 512:1024],
                                     start=(f == 0), stop=(f == 3))
                for f in range(4):
                    nc.tensor.matmul(pv[:], hF[:, ts(f, 128)],
                                     swt[f][:, 1024:1536],
                                     start=(f == 0), stop=(f == 3))
                for f in range(4):
                    nc.tensor.matmul(pbt[:], hF[:, ts(f, 128)],
                                     swt[f][:, 1536:1544],
                                     start=(f == 0), stop=(f == 3))

                # softmax(q), softmax(k); sigmoid(beta) via Exp (keeps the
                # ACT table on Exp); kb = beta*k; R built per head
                stat = ap.tile([128, 40], f32, tag="smstat", name="smstat")
                q_exp = ap.tile([128, 512], bft, tag="q_exp", name="q_exp")
                k_exp = ap.tile([128, 512], bft, tag="k_exp", name="k_exp")
                q_sm = ap.tile([128, 512], bft, tag="q_sm", name="q_sm")
                k_sm = ap.tile([128, 512], bft, tag="k_sm", name="k_sm")
                kb_sm = ap.tile([128, 512], bft, tag="kb_sm", name="kb_sm")
                R = ap.tile([128, 1024], bft, tag="R", name="R")
                bet = stat[:, 16:24]
                nc.scalar.activation(bet, pbt[:], act.Exp, scale=-1.0)
                nc.vector.tensor_scalar_add(bet, bet, 1.0)
                nc.vector.reciprocal(bet, bet)                    # sigmoid
                nc.scalar.activation(q_exp[:], pq[:], act.Exp)
                nc.scalar.activation(k_exp[:], pk[:], act.Exp)
                nc.vector.tensor_reduce(
                    stat[:, 0:8], q_exp[:].rearrange("p (h d) -> p h d", h=8),
                    mybir.AxisListType.X, alu.add)
                nc.vector.tensor_reduce(
                    stat[:, 8:16], k_exp[:].rearrange("p (h d) -> p h d", h=8),
                    mybir.AxisListType.X, alu.add)
                nc.vector.reciprocal(stat[:, 24:32], stat[:, 0:8])   # 1/sum q
                nc.vector.reciprocal(stat[:, 32:40], stat[:, 8:16])  # 1/sum k
                nc.vector.tensor_mul(stat[:, 8:16], bet, stat[:, 32:40])  # b/s

                def hb(a):      # [128,8] -> [128,8,64] broadcast view
                    return a.unsqueeze(2).broadcast_to([128, 8, 64])

                def hd(a):      # [128,512] -> [128,8,64]
                    return a.rearrange("p (h d) -> p h d", h=8)

                nc.vector.tensor_mul(hd(q_sm[:]), hd(q_exp[:]),
                                     hb(stat[:, 24:32]))
                nc.vector.tensor_mul(hd(kb_sm[:]), hd(k_exp[:]),
                                     hb(stat[:, 8:16]))
                nc.vector.tensor_mul(hd(k_sm[:]), hd(k_exp[:]),
                                     hb(stat[:, 32:40]))
                v_sb = ap.tile([128, 512], bft, tag="v_sb", name="v_sb")
                nc.scalar.copy(v_sb[:], pv[:])
                # R: even heads [kb | bv], odd heads [bv | kb]
                R3 = R[:].rearrange("p (h c) -> p h c", c=128)
                for par in (0, 1):
                    vc, kc = (64, 0) if par == 0 else (0, 64)
                    nc.vector.tensor_mul(
                        R3[:, par:8:2, vc:vc + 64],
                        hd(v_sb[:])[:, par:8:2, :],
                        hb(bet)[:, par:8:2, :])
                    nc.vector.tensor_copy(R3[:, par:8:2, kc:kc + 64],
                                          hd(kb_sm[:])[:, par:8:2, :])

                if dbg and li == 0 and c == 0:
                    nc.gpsimd.dma_start(dbg_q[:], q_sm[:])
                    nc.gpsimd.dma_start(dbg_k[:], k_sm[:])
                    nc.gpsimd.dma_start(dbg_kb[:], kb_sm[:])
                    nc.sync.dma_start(dbg_bet[:], bet)
                # FM transposes: head pair p -> partitions (h%2)*64.
                # F2 holds [kb-pair | q-pair] per 256-col group so the
                # SbT and (QK^T)^T matmuls share one N=256 rhs.
                kF = ap.tile([128, 512], bft, tag="kF", name="kF")
                F2 = ap.tile([128, 1024], bft, tag="F2", name="F2")
                transpose4([k_sm[:, ts(p, 128)] for p in range(4)], kF)
                for g in range(2):
                    transpose4([kb_sm[:, ts(2 * g, 128)],
                                q_sm[:, ts(2 * g, 128)],
                                kb_sm[:, ts(2 * g + 1, 128)],
                                q_sm[:, ts(2 * g + 1, 128)]],
                               F2[:, ts(g, 512)])

                for h in range(8):
                    bh = (h % 2) * 64          # FM base partition
                    oh = 64 - bh               # the other half
                    p = h // 2
                    qFh = F2[bh:bh + 64, 256 * p + 128:256 * p + 256]
                    kFh = kF[bh:bh + 64, ts(p, 128)]
                    kbFh = F2[bh:bh + 64, 256 * p:256 * p + 128]
                    kTM = k_sm[:, ts(h, 64)]
                    Rh = R[:, ts(h, 128)]
                    tkc, tvc = (0, 64) if h % 2 == 0 else (64, 0)

                    pss = ps_ss.tile([128, 384], f32, tag="ss", name="ss")
                    nc.tensor.matmul(pss[:, 0:128], kbFh, kFh)      # Sb
                    # one N=256 matmul: [Sb^T | (QK^T)^T]
                    nc.tensor.matmul(pss[:, 128:384], kFh,
                                     F2[bh:bh + 64, 256 * p:256 * p + 256])
                    AAM = ip.tile([128, 384], bft, tag="AAM", name="AAM")
                    nc.vector.tensor_mul(AAM[:], pss[:, 0:384], mall[:])
                    A = AAM[:, 0:128]
                    At = AAM[:, 128:256]
                    Mt = AAM[:, 256:384]

                    wk = ps_wrk.tile([128, 512], f32, tag="wrk", name="wrk")
                    # Neumann order 4: X = (I - A)(I + A^2) R
                    nc.tensor.matmul(wk[:, 0:128], A, At)           # (A^2)^T
                    A2t = ip.tile([128, 128], bft, tag="A2t", name="A2t")
                    nc.scalar.copy(A2t[:], wk[:, 0:128])
                    Y = ip.tile([128, 128], bft, tag="Y", name="Y")
                    nc.tensor.matmul(wk[:, 128:256], A2t[:], Rh)    # A^2 R
                    nc.vector.tensor_add(Y[:], wk[:, 128:256], Rh)
                    X = ip.tile([128, 128], bft, tag="X", name="X")
                    nc.tensor.matmul(wk[:, 256:384], At, Y[:])      # A Y
                    nc.vector.tensor_sub(X[:], Y[:], wk[:, 256:384])

                    # M X (transposed): (M Tk)^T -> cols 0:128 (A2t's cols,
                    # long consumed); (M Tv)^T + O^T accum -> cols 384:512.
                    # Disjoint column ranges keep each accumulation group's
                    # columns private until it closes.
                    nc.tensor.matmul(wk[tkc:tkc + 64, 0:128],
                                     X[:, tkc:tkc + 64], Mt)
                    P = ip.tile([128, 128], bft, tag="P", name="P")
                    nc.vector.tensor_sub(P[bh:bh + 64, :], qFh,
                                         wk[bh:bh + 64, 0:128])
                    # O^T = W0 P^T + (M Tv)^T; the group opens and closes on
                    # two adjacent PE ops so no other read of this bank can
                    # land inside it.
                    if c > 0:
                        nc.tensor.matmul(wk[oh:oh + 64, 384:512],
                                         wtb[h][bh:bh + 64, :],
                                         P[bh:bh + 64, :],
                                         start=True, stop=False)
                    nc.tensor.matmul(wk[tvc:tvc + 64, 384:512],
                                     X[:, tvc:tvc + 64], Mt,
                                     start=(c == 0), stop=True)
                    nc.scalar.copy(attn[p][oh:oh + 64, ts(c, 128)],
                                   wk[oh:oh + 64, 384:512])

                    if c < NT - 1:
                        # W recurrence (transposed carry Wt [e,d]):
                        # Wt' = Wt + K^T Tv - G'^T Wt,  G' = Tk^T K
                        nGb = ip.tile([128, 64], bft, tag="nG", name="nG")
                        nc.tensor.matmul(wk[bh:bh + 64, 128:192],
                                         X[:, tkc:tkc + 64], kTM)   # G'
                        nc.scalar.mul(nGb[bh:bh + 64, :],
                                      wk[bh:bh + 64, 128:192], -1.0)
                        nc.tensor.matmul(wk[bh:bh + 64, 192:256], kTM,
                                         X[:, tvc:tvc + 64],
                                         start=True, stop=(c == 0))
                        if c > 0:
                            nc.tensor.matmul(wk[bh:bh + 64, 192:256],
                                             nGb[bh:bh + 64, :],
                                             wtb[h][bh:bh + 64, :],
                                             start=False, stop=True)
                        nwtf = sp.tile([128, 64], f32, tag=f"wtf{h}", name=f"wtf{h}")
                        nwtb = sp.tile([128, 64], bft, tag=f"wtb{h}", name=f"wtb{h}")
                        if c > 0:
                            nc.vector.tensor_add(nwtf[bh:bh + 64, :],
                                                 wk[bh:bh + 64, 192:256],
                                                 wtf[h][bh:bh + 64, :])
                        else:
                            nc.vector.tensor_copy(nwtf[bh:bh + 64, :],
                                                  wk[bh:bh + 64, 192:256])
                        nc.scalar.copy(nwtb[bh:bh + 64, :],
                                       nwtf[bh:bh + 64, :])
                        wtf[h] = nwtf
                        wtb[h] = nwtb

                # out proj for this chunk + residual add
                po = ps_big.tile([128, HID], f32, tag="big", name="big")
                for g in range(4):
                    nc.tensor.matmul(po[:], attn[g][:, ts(c, 128)], owt[g][:],
                                     start=(g == 0), stop=(g == 3))
                nc.vector.tensor_add(rt[c][:], rt[c][:], po[:])
            if dbg and li == 0:
                for g in range(4):
                    nc.gpsimd.dma_start(dbg_attn[g], attn[g][:])
                for i2 in range(NT):
                    nc.sync.dma_start(dbg_r1[ts(i2, 128), :], rt[i2][:])

        def ffn():
            BS = min(4, NT)            # token tiles per block
            for blk in range(NT // BS):
                h2F = [ffp.tile([128, BS * 128], bft, tag=f"h2F{f}",
                                name=f"h2F{f}") for f in range(4)]
                stf = ln_stats([rt[blk * BS + t] for t in range(BS)])
                h2t = []
                for t in range(BS):
                    i = blk * BS + t
                    h2 = ap.tile([128, HID], bft, tag="h2_tm", name="h2_tm",
                                 bufs=6)
                    ln_apply(stf, t, rt[i], h2)
                    h2t.append(h2)
                for f in range(4):
                    transpose4([h2t[t][:, ts(f, 128)] for t in range(BS)],
                               h2F[f], gcol=(lng[1], lnb[1], f))
                rh = [ffp.tile([128, BS * 128], bft, tag=f"rh{ff}",
                               name=f"rh{ff}") for ff in range(16)]
                for ff in range(16):
                    pw = ps_big.tile([128, BS * 128], f32, tag="big",
                                     name="big")
                    for f in range(4):
                        nc.tensor.matmul(pw[:], w1t[f][:, ts(ff, 128)],
                                         h2F[f][:],
                                         start=(f == 0), stop=(f == 3))
                    nc.scalar.activation(rh[ff][:], pw[:], act.Relu,
                                         bias=b1c[ff][:])
                for t in range(BS):
                    i = blk * BS + t
                    pw = ps_big.tile([128, HID], f32, tag="big", name="big")
                    for ff in range(16):
                        nc.tensor.matmul(pw[:], rh[ff][:, ts(t, 128)],
                                         w2t[ff][:],
                                         start=(ff == 0), stop=(ff == 15))
                    nc.vector.tensor_add(rt[i][:], rt[i][:], pw[:])
                    nc.vector.tensor_add(rt[i][:], rt[i][:], b2[:])

        fast_ff(0, 0)
        ffn()
        if dbg:
            for i2 in range(NT):
                nc.sync.dma_start(dbg_r2[ts(i2, 128), :], rt[i2][:])
        fast_ff(1, 2)
        for i in range(NT):
            nc.sync.dma_start(out_d[ts(i, 128), :], rt[i][:])

    nc.compile()
    return nc


def _prep_shared(ip_w, ip_b, fw_ln_g, fw_ln_b, fw_slow_w, fw_out_w,
                 ff_ln_g, ff_ln_b, ff_w1, ff_b1, ff_w2, ff_b2):
    """Host-side preprocessing of weights shared by all cores."""
    ones = np.ones((128, 1), np.float32)

    # slow_w rows permuted to [all q | all k | all v | all beta]
    def perm_slow(w):
        w = w.reshape(H, 3 * D + 1, HID)
        q = w[:, 0:D].reshape(H * D, HID)
        k = w[:, D:2 * D].reshape(H * D, HID)
        v = w[:, 2 * D:3 * D].reshape(H * D, HID)
        b = w[:, 3 * D].reshape(H, HID)
        return np.concatenate([q, k, v, b], 0).T.copy()     # [HID, 1544]

    swT = np.stack([perm_slow(fw_slow_w[0]), perm_slow(fw_slow_w[1])])

    # out_w cols permuted: attn row (g,half,d) -> head h = 2g + (1-half//64)
    hdmap = np.empty(HID, np.int64)
    for row in range(HID):
        g, rr = row // 128, row % 128
        half, d = rr // 64, rr % 64
        h = 2 * g + (1 - half)
        hdmap[row] = h * 64 + d
    owT = np.stack([fw_out_w[0].T[hdmap].copy(), fw_out_w[1].T[hdmap].copy()])

    tri = np.tril(np.ones((128, 128), np.float32), -1)
    masks = np.stack([tri, tri.T, np.triu(np.ones((128, 128), np.float32))])

    return {
        "ipwT": ip_w.T.astype(bfnp).copy(),
        "ipb_bc": (ones * ip_b[None, :]).astype(np.float32),
        "swT": swT.astype(bfnp),
        "owT": owT.astype(bfnp),
        "w1T": ff_w1.T.astype(bfnp).copy(),
        "b1c": ff_b1.reshape(16, 128, 1).astype(np.float32),
        "w2T": ff_w2.T.astype(bfnp).copy(),
        "b2_bc": (ones * ff_b2[None, :]).astype(np.float32),
        "lng_bc": np.stack([fw_ln_g[0], ff_ln_g, fw_ln_g[1]])
        .reshape(3, 4, 128).transpose(0, 2, 1).astype(np.float32).copy(),
        "lnb_bc": np.stack([fw_ln_b[0], ff_ln_b, fw_ln_b[1]])
        .reshape(3, 4, 128).transpose(0, 2, 1).astype(np.float32).copy(),
        "masks": masks,
        "ident": np.eye(128).astype(bfnp),
    }


def kernel(x, ip_w, ip_b, fw_ln_g, fw_ln_b, fw_slow_w, fw_out_w,
           ff_ln_g, ff_ln_b, ff_w1, ff_b1, ff_w2, ff_b2):
    import concourse.bacc as bacc
    from concourse.bass_utils import run_bass_kernel_spmd

    if "nc" not in _CACHED:
        _CACHED["nc"] = build(bacc.Bacc)
    nc = _CACHED["nc"]

    shared = _prep_shared(ip_w, ip_b, fw_ln_g, fw_ln_b, fw_slow_w, fw_out_w,
                          ff_ln_g, ff_ln_b, ff_w1, ff_b1, ff_w2, ff_b2)
    in_maps = []
    for b in range(N_CORES):
        m = dict(shared)
        m["xT"] = x[b].T.astype(bfnp).copy()
        in_maps.append(m)

    res = run_bass_kernel_spmd(nc, in_maps, list(range(N_CORES)))
    out = np.stack([res.results[b]["out"] for b in range(N_CORES)])
    return out.astype(np.float32)
